# revision 36
# baseline (speedup 1.0000x reference)
"""Trainium2 Bass kernel for CausalCrossConditionalSelfAttention.

Reference semantics (B=2, T=2560, C=768, H=12, hd=64, t=T//10=256):
  q/k/v = x @ W{q,k,v}.T + b{q,k,v}           (per-head slices of C)
  att   = softmax(mask(q k^T / sqrt(hd)))      mask: (i%256) >= (j%256)
  y     = (att @ v) @ Wp.T + bp

Key trick: sort positions by residue r = i%256 (host-side permute of x;
inverse-permute of the output).  In sorted coordinates (s = r*10 + tile)
the mask becomes BLOCK-CAUSAL: visible iff s_q//10 >= s_k//10, so only
~half of the TxT score slab is ever computed (vs 75% for the baseline's
quadrant skip).  The staircase mask inside each diagonal 128x128 block is
rank-13: it is folded into the score matmul as 13 extra contraction rows
(A-rows on the K side carrying -1e9, B-rows on the Q side carrying 0/1),
so no separate mask-add matmuls are needed.  A 14th extra row handles the
bq.k bias term exactly (q.bk and bq.bk are softmax-invariant, dropped).

Sharding: 8 cores = 2 batches x 4 head-groups (3 heads each).  Each core
returns a partial pre-projection out^T [768, 2560] (sorted coords); the
host sums the 4 head-group partials per batch, adds const (bp + Wp bv),
and inverse-permutes.

Softmax denominator rides as a 65th 'ones' column of V through the AV
matmul.  Normalization is a 4-stage deferred pipeline (one stage popped
per score slab): av->SBUF copy (frees the PSUM slot early), denominator
row -> DRAM -> [128,4] reshaped readback (cheap DVE reciprocal; the
[1,512] shape costs ~3.3us), reciprocal -> DRAM -> [64,512] stride-0
partition-broadcast readback, then a DVE multiply.  The DMA hops
alternate between the sync and gpsimd queues so no single in-order
queue head blocks on the chain latency.  Partial outputs are bf16
(summed in f32 on the host).
"""

import numpy as np

B, T, C = 2, 2560, 768
H, HD = 12, 64
HPG = 3            # heads per group (core)
CW = HPG * HD      # 192
NKC = T // 128     # 20 key chunks of 128
NQT = T // 512     # 5 query tiles of 512
N_CORES = 8
NMR = 13           # staircase mask rank
QR = 64 + 1 + NMR  # q/k tile rows: 64 data + bias/ones + 13 mask rows

_CACHE = {}


def _split_multi_waits(nc, maxw=1):
    """walrus in this container rejects >1 sync wait per instruction;
    split extra waits onto preceding NOPs on the same engine."""
    import concourse.mybir as mybir
    for f in nc.m.functions:
        for bb in f.blocks:
            newlist = []
            for ins in bb.instructions:
                si = ins.sync_info
                if si is not None and si.on_wait and len(si.on_wait) > maxw:
                    waits = list(si.on_wait)
                    chunks = [waits[i:i + maxw] for i in range(0, len(waits), maxw)]
                    for ch in chunks[:-1]:
                        newlist.append(mybir.InstNoOp(
                            name=f"WSPLIT-{nc.next_id()}",
                            engine=ins.engine,
                            sync_info=mybir.SyncInfo(on_wait=list(ch), on_update=[]),
                            text_hint="wait_split",
                        ))
                    ins.sync_info = mybir.SyncInfo(
                        on_wait=list(chunks[-1]), on_update=list(si.on_update))
                newlist.append(ins)
            bb.instructions = newlist
    return nc


def build_program():
    import concourse.bass as bass
    import concourse.mybir as mybir
    import concourse.tile as tile

    f32 = mybir.dt.float32
    bf16 = mybir.dt.bfloat16
    AF = mybir.ActivationFunctionType

    nc = bass.Bass()
    xT = nc.dram_tensor("xT", [C, T], bf16, kind="ExternalInput")
    wqk = nc.dram_tensor("wqk", [C, 384], bf16, kind="ExternalInput")
    cq = nc.dram_tensor("cq", [NMR + 1, T], bf16, kind="ExternalInput")
    ck = nc.dram_tensor("ck", [HPG, NMR + 1, T], bf16, kind="ExternalInput")
    wv = nc.dram_tensor("wv", [C, CW], bf16, kind="ExternalInput")
    wp01 = nc.dram_tensor("wp01", [128, C], bf16, kind="ExternalInput")
    wp2 = nc.dram_tensor("wp2", [64, C], bf16, kind="ExternalInput")
    out = nc.dram_tensor("out", [C, T], bf16, kind="ExternalOutput")
    rcpb = nc.dram_tensor("rcpb", [HPG * NQT, 512], f32)
    rcpb2 = nc.dram_tensor("rcpb2", [HPG * NQT, 512], f32)

    with tile.TileContext(nc) as tc:
        with tc.tile_pool(name="persist", bufs=1) as persist, \
             tc.tile_pool(name="work", bufs=2) as work, \
             tc.tile_pool(name="psum", bufs=2, space="PSUM") as psum:

            # ---------------- load inputs ----------------
            # batched multi-dim DMAs (one dispatch each): the sync queue
            # dispatches serially at ~0.6us per DMA, so fewer is faster.
            # order matters: the first proj matmul needs wqk + xt[qt0].
            wqk_sb = persist.tile([128, 6, 384], bf16)
            wv_sb = persist.tile([128, 6, CW], bf16)
            xt_sb = persist.tile([128, 6, T], bf16)       # x^T, 6 chunks of C

            def _xt_in(qt):
                return bass.AP(tensor=xT, offset=qt * 512,
                               ap=[[T, 128], [128 * T, 6], [1, 512]])
            nc.sync.dma_start(out=xt_sb[:, :, 0:512], in_=_xt_in(0))
            nc.sync.dma_start(
                out=wqk_sb,
                in_=bass.AP(tensor=wqk, offset=0,
                            ap=[[384, 128], [128 * 384, 6], [1, 384]]))
            nc.sync.dma_start(
                out=wv_sb,
                in_=bass.AP(tensor=wv, offset=0,
                            ap=[[CW, 128], [128 * CW, 6], [1, CW]]))
            for qt in range(1, NQT):
                nc.sync.dma_start(out=xt_sb[:, :, qt * 512:(qt + 1) * 512],
                                  in_=_xt_in(qt))
            wp01_sb = persist.tile([128, C], bf16)
            nc.sync.dma_start(out=wp01_sb, in_=wp01[:, :])
            wp2_sb = persist.tile([64, C], bf16)
            nc.sync.dma_start(out=wp2_sb, in_=wp2[:, :])

            # q' / k' tiles: rows 0:64 = projections (device), row 64 =
            # ones / bq.k bias row, rows 65:78 = staircase mask rows (host)
            qpr = [persist.tile([QR, T], bf16, name=f"qpr{h}")
                   for h in range(HPG)]
            kpr = [persist.tile([QR, T], bf16, name=f"kpr{h}")
                   for h in range(HPG)]
            for h in range(HPG):
                nc.sync.dma_start(out=qpr[h][64:QR, :], in_=cq[:, :])
                nc.sync.dma_start(out=kpr[h][64:QR, :], in_=ck[h])

            # v natural layout + ones column: per head h cols
            # [65h .. 65h+63] = V_h, col 65h+64 = 1.0
            v_sb = persist.tile([128, NKC, HPG * 65], bf16)
            v_r = v_sb.rearrange("p n (h c) -> p n h c", c=65)
            nc.vector.memset(v_r[:, :, :, 64], 1.0)

            ynorm01 = persist.tile([128, T], bf16)   # heads 0,1 normalized y
            ynorm2 = persist.tile([64, T], bf16)     # head 2

            # ---------------- phase 1: projections ----------------
            # qk groups g: 0=[Qh0|Qh1] 1=[Kh0|Kh1] 2=[Qh2|Kh2], each 128 wide
            # interleaved with v chunks to hide v's LDWEIGHTS-bound matmuls
            qk_dst = [(qpr[0], qpr[1]), (kpr[0], kpr[1]), (qpr[2], kpr[2])]
            for qt in range(NQT):
                for g in range(4):
                    tch = 4 * qt + g
                    do_qk = g < 3
                    if do_qk:
                        qk_ps = psum.tile([128, 512], f32, tag="pv", bufs=3,
                                          name="qk_ps")
                    v_ps = psum.tile([128, 512], f32, tag="pv", bufs=3,
                                     name="v_ps")
                    for c in range(6):
                        if do_qk:
                            nc.tensor.matmul(
                                qk_ps,
                                lhsT=wqk_sb[:, c, g * 128:(g + 1) * 128],
                                rhs=xt_sb[:, c, qt * 512:(qt + 1) * 512],
                                start=(c == 0), stop=(c == 5))
                        nc.tensor.matmul(
                            v_ps[:, :CW],
                            lhsT=xt_sb[:, c, tch * 128:(tch + 1) * 128],
                            rhs=wv_sb[:, c, :],
                            start=(c == 0), stop=(c == 5))
                    if do_qk:
                        d0, d1 = qk_dst[g]
                        nc.vector.tensor_copy(
                            d0[0:64, qt * 512:(qt + 1) * 512], qk_ps[0:64, :])
                        nc.vector.tensor_copy(
                            d1[0:64, qt * 512:(qt + 1) * 512], qk_ps[64:128, :])
                    nc.vector.tensor_copy(
                        v_r[:, tch, :, 0:64],
                        v_ps[:, :CW].rearrange("p (h c) -> p h c", h=HPG))

            # ---------------- phase 2: attention ----------------
            # Per (h, qt): sorted-causal visible kcs = 0 .. 4qt+3.
            # kc < 4qt fully visible; kc = 4qt+i diagonal with local query
            # offset 128i (mask rows used on the first 128 cols).
            pending = []

            def _emit_pj(qt_p):
                for m in range(6):
                    pj_ps = psum.tile([128, 512], f32, tag="pv", bufs=3,
                                      name="pj_ps")
                    nc.tensor.matmul(
                        pj_ps, lhsT=wp01_sb[:, m * 128:(m + 1) * 128],
                        rhs=ynorm01[:, qt_p * 512:(qt_p + 1) * 512],
                        start=True, stop=False)
                    nc.tensor.matmul(
                        pj_ps, lhsT=wp2_sb[:, m * 128:(m + 1) * 128],
                        rhs=ynorm2[:, qt_p * 512:(qt_p + 1) * 512],
                        start=False, stop=True)
                    pj_sb = work.tile([128, 512], bf16, tag="pj", bufs=4,
                                      name="pj_sb")
                    nc.vector.tensor_copy(pj_sb, pj_ps)
                    nc.gpsimd.dma_start(
                        out=out[m * 128:(m + 1) * 128,
                                qt_p * 512:(qt_p + 1) * 512],
                        in_=pj_sb)

            def _norm_a(item):
                """Copy av (psum) to SBUF -- frees the psum slot early --
                and kick the denominator row to DRAM + its [128,4] reshaped
                readback (the [1,512]-shaped DVE reciprocal costs ~3.3us,
                the [128,4] one ~0.2us)."""
                av_p, h_p, qt_p = item
                slot = h_p * NQT + qt_p
                av_sb = work.tile([65, 512], f32, tag="avs", bufs=4,
                                  name="av_sb")
                nc.vector.tensor_copy(av_sb, av_p)
                d128 = work.tile([128, 4], f32, tag="d128", bufs=3,
                                 name="d128")
                nc.gpsimd.dma_start(out=d128, in_=av_sb[64:65, :])
                return (av_sb, d128, h_p, qt_p)

            def _norm_b(item):
                av_sb, d128, h_p, qt_p = item
                slot = h_p * NQT + qt_p
                rcp = work.tile([128, 4], f32, tag="rcp4", bufs=3, name="rcp")
                nc.vector.reciprocal(rcp, d128)
                r4b = bass.AP(tensor=rcpb2, offset=slot * 512,
                              ap=[[4, 128], [1, 4]])
                nc.sync.dma_start(out=r4b, in_=rcp)
                return (av_sb, h_p, qt_p)

            def _norm_c(item):
                av_sb, h_p, qt_p = item
                slot = h_p * NQT + qt_p
                bc_sb = work.tile([64, 512], f32, tag="bc", bufs=3,
                                  name="bc_sb")
                bcast_in = bass.AP(tensor=rcpb2, offset=slot * 512,
                                   ap=[[0, 64], [1, 512]])
                nc.gpsimd.dma_start(out=bc_sb, in_=bcast_in)
                return (av_sb, bc_sb, h_p, qt_p)

            def _norm_d(item):
                av_sb, bc_sb, h_p, qt_p = item
                if h_p == 0:
                    dst = ynorm01[0:64, qt_p * 512:(qt_p + 1) * 512]
                elif h_p == 1:
                    dst = ynorm01[64:128, qt_p * 512:(qt_p + 1) * 512]
                else:
                    dst = ynorm2[:, qt_p * 512:(qt_p + 1) * 512]
                nc.gpsimd.tensor_mul(dst, av_sb[0:64, :], bc_sb)
                return (h_p, qt_p)

            def _pop_pending():
                if not pending:
                    return
                stage, item = pending.pop(0)
                if stage == 0:
                    pending.append((1, _norm_a(item)))
                elif stage == 1:
                    pending.append((2, _norm_b(item)))
                elif stage == 2:
                    pending.append((3, _norm_c(item)))
                elif stage == 3:
                    h_p, qt_p = _norm_d(item)
                    if h_p == HPG - 1:
                        pending.append((4, qt_p))
                else:
                    _emit_pj(item)

            def _zb(j):
                return 128 * j - (128 * j) % 10

            for qt in range(NQT):
                q0 = qt * 512
                # regions: (kc, w0, w1, masked) with [w0, w1) window-relative.
                # Key chunk j: masked zone queries [zb(j), zb(j+1)), fully
                # visible queries >= zb(j+1), invisible below zb(j).
                regions = []
                for j in range(NKC + 1):
                    if j >= NKC or _zb(j) >= q0 + 512:
                        break
                    mlo = max(q0, _zb(j))
                    mhi = min(q0 + 512, _zb(j + 1))
                    if mhi > mlo:
                        regions.append((j, mlo - q0, mhi - q0, True))
                    flo = max(q0, _zb(j + 1))
                    if q0 + 512 > flo:
                        regions.append((j, flo - q0, 512, False))
                # pack into slabs of <=1024 sc cols, splitting at the 512-col
                # psum bank boundaries (a matmul may not cross a bank)
                slabs = []
                cur, cur_cols = [], 0
                for kc, w0, w1, masked in regions:
                    while w0 < w1:
                        if cur_cols == 1024:
                            slabs.append(cur)
                            cur, cur_cols = [], 0
                        bank_room = 512 - cur_cols % 512
                        take = min(w1 - w0, bank_room)
                        cur.append((kc, w0, w0 + take, cur_cols, masked))
                        cur_cols += take
                        w0 += take
                if cur:
                    slabs.append(cur)
                last_piece = slabs[-1][-1]
                pieces0 = slabs[0][0]

                for h in range(HPG):
                    av = psum.tile([65, 512], f32, tag="pv", bufs=3, name="av")
                    av_todo = []

                    def _emit_av(args, h=h, av=av, p0=pieces0,
                                 pl=last_piece):
                        pt, pieces = args
                        for piece in pieces:
                            kc, w0, w1, soff, masked = piece
                            nc.tensor.matmul(
                                av[:, w0:w1],
                                lhsT=v_sb[:, kc, 65 * h:65 * h + 65],
                                rhs=pt[:, soff:soff + (w1 - w0)],
                                start=(piece == p0),
                                stop=(piece == pl),
                                skip_group_check=True)

                    def _emit_slab(pieces, h=h):
                        sc = psum.tile([128, 1024], f32, tag="sc", name="sc")
                        total = 0
                        bank_started = set()
                        for kc, w0, w1, soff, masked in pieces:
                            kcol = kc * 128
                            rows = QR if masked else 65
                            st = (soff // 512) not in bank_started
                            bank_started.add(soff // 512)
                            nc.tensor.matmul(
                                sc[:, soff:soff + (w1 - w0)],
                                lhsT=kpr[h][0:rows, kcol:kcol + 128],
                                rhs=qpr[h][0:rows, q0 + w0:q0 + w1],
                                start=st, stop=True,
                                skip_group_check=True)
                            total = soff + (w1 - w0)
                        pt = work.tile([128, 1024], bf16, tag="pt", bufs=3,
                                       name="pt")
                        nc.scalar.activation(pt[:, :total], sc[:, :total],
                                             AF.Exp, scale=0.125)
                        return pt

                    for pieces in slabs:
                        _pop_pending()
                        pt = _emit_slab(pieces)
                        av_todo.append((pt, pieces))
                        if len(av_todo) > 1:
                            _emit_av(av_todo.pop(0))
                    while av_todo:
                        _emit_av(av_todo.pop(0))
                    pending.append((0, (av, h, qt)))
            while pending:
                _pop_pending()

    _split_multi_waits(nc)
    return nc


def get_program():
    if "nc" not in _CACHE:
        _CACHE["nc"] = build_program()
    return _CACHE["nc"]


def _perm():
    # device position s <-> original position perm[s] = (s%10)*256 + s//10
    s = np.arange(T)
    return (s % 10) * 256 + s // 10


def _mask_rows():
    """A rows (key side, carry -1e9, base = own 128-chunk) and B rows
    (query side 0/1, base = the key chunk whose visibility zone holds q)."""
    s = np.arange(T)
    r = s // 10                        # residue of sorted position
    baseA = (128 * (s // 128)) // 10   # key chunk base residue
    locA = r - baseA
    z = (10 * (s // 10) + 9) // 128    # zone: key chunk containing q's bound
    baseB = (128 * z) // 10
    locB = r - baseB
    ells = np.arange(1, NMR + 1)[:, None]
    A = np.where(locA[None, :] >= ells, np.float32(-1e9), np.float32(0.0))
    Bm = (locB[None, :] < ells).astype(np.float32)
    return A, Bm


def make_in_maps(x, Wk, bk, Wq, bq, Wv, bv, Wp, bp):
    import ml_dtypes
    b16 = ml_dtypes.bfloat16
    x = np.asarray(x, dtype=np.float32)
    Wk = np.asarray(Wk, dtype=np.float32)
    Wq = np.asarray(Wq, dtype=np.float32)
    Wv = np.asarray(Wv, dtype=np.float32)
    Wp = np.asarray(Wp, dtype=np.float32)
    bq_f = np.asarray(bq, dtype=np.float32)
    perm = _perm()
    A, Bm = _mask_rows()
    cq = np.concatenate([np.ones((1, T), np.float32), Bm], axis=0)

    in_maps = []
    for core in range(N_CORES):
        b, g = divmod(core, 4)
        h0 = g * HPG
        r = slice(h0 * HD, (h0 + HPG) * HD)      # 192 head dims
        xs = x[b][perm]                          # sorted positions [T, C]
        xt = np.ascontiguousarray(xs.T)
        wq_g = Wq[r]                             # [192, 768]
        wk_g = Wk[r]
        # wqk cols: [Qh0|Qh1(128) | Kh0|Kh1(128) | Qh2|Kh2(128)]
        wqk = np.concatenate(
            [wq_g[:128].T, wk_g[:128].T, wq_g[128:].T, wk_g[128:].T],
            axis=1).astype(np.float32)
        # k-side extra rows: row 0 = bq.k_j = xs @ (Wk_h^T bq_h), rows 1:14 = A
        ck = np.empty((HPG, NMR + 1, T), np.float32)
        for h in range(HPG):
            hb = slice((h0 + h) * HD, (h0 + h + 1) * HD)
            khat = xs @ (Wk[hb].T @ bq_f[hb])    # [T]
            ck[h, 0] = khat
            ck[h, 1:] = A
        wv_g = np.ascontiguousarray(Wv[r].T).astype(np.float32)
        wp_g = Wp[:, r]                          # [768, 192]
        wp01 = np.ascontiguousarray(wp_g[:, 0:128].T)
        wp2 = np.ascontiguousarray(wp_g[:, 128:192].T)
        in_maps.append({
            "xT": np.ascontiguousarray(xt).astype(b16),
            "wqk": np.ascontiguousarray(wqk).astype(b16),
            "cq": cq.astype(b16),
            "ck": ck.astype(b16),
            "wv": wv_g.astype(b16),
            "wp01": wp01.astype(b16),
            "wp2": wp2.astype(b16),
        })
    return in_maps


def kernel(x, Wk, bk, Wq, bq, Wv, bv, Wp, bp):
    from concourse.bass_utils import run_bass_kernel_spmd
    nc = get_program()
    in_maps = make_in_maps(x, Wk, bk, Wq, bq, Wv, bv, Wp, bp)
    res = run_bass_kernel_spmd(nc, in_maps, list(range(N_CORES)))
    Wp_np = np.asarray(Wp, dtype=np.float32)
    const = (np.asarray(bp, dtype=np.float32)
             + Wp_np @ np.asarray(bv, dtype=np.float32))   # [768]
    perm = _perm()
    out = np.empty((B, T, C), dtype=np.float32)
    for b in range(B):
        acc = res.results[b * 4 + 0]["out"].astype(np.float32).copy()
        for g in range(1, 4):
            acc += res.results[b * 4 + g]["out"]
        out[b][perm] = acc.T + const[None, :]
    return out


# revision 37
# speedup vs baseline: 1.1143x; 1.1143x over previous
"""Trainium2 Bass kernel for CausalCrossConditionalSelfAttention.

Reference semantics (B=2, T=2560, C=768, H=12, hd=64, t=T//10=256):
  q/k/v = x @ W{q,k,v}.T + b{q,k,v}           (per-head slices of C)
  att   = softmax(mask(q k^T / sqrt(hd)))      mask: (i%256) >= (j%256)
  y     = (att @ v) @ Wp.T + bp

Key trick: sort positions by residue r = i%256 (host-side permute of x;
inverse-permute of the output).  In sorted coordinates (s = r*10 + tile)
the mask becomes BLOCK-CAUSAL: visible iff s_q//10 >= s_k//10, so only
~half of the TxT score slab is ever computed (vs 75% for the baseline's
quadrant skip).  The staircase mask inside each diagonal 128x128 block is
rank-13: it is folded into the score matmul as 13 extra contraction rows
(A-rows on the K side carrying -1e9, B-rows on the Q side carrying 0/1),
so no separate mask-add matmuls are needed.  A 14th extra row handles the
bq.k bias term exactly (q.bk and bq.bk are softmax-invariant, dropped).

Sharding: 8 cores = 2 batches x 4 head-groups (3 heads each).  Each core
returns a partial pre-projection out^T [768, 2560] (sorted coords); the
host sums the 4 head-group partials per batch, adds const (bp + Wp bv),
and inverse-permutes.

Softmax denominator rides as a 65th 'ones' column of V through the AV
matmul.  Normalization is a 4-stage deferred pipeline (one stage popped
per score slab): av->SBUF copy (frees the PSUM slot early), denominator
row -> DRAM -> [128,4] reshaped readback (cheap DVE reciprocal; the
[1,512] shape costs ~3.3us), reciprocal -> DRAM -> [64,512] stride-0
partition-broadcast readback, then a DVE multiply.  The DMA hops
alternate between the sync and gpsimd queues so no single in-order
queue head blocks on the chain latency.  Partial outputs are bf16
(summed in f32 on the host).
"""

import numpy as np

B, T, C = 2, 2560, 768
H, HD = 12, 64
HPG = 3            # heads per group (core)
CW = HPG * HD      # 192
NKC = T // 128     # 20 key chunks of 128
NQT = T // 512     # 5 query tiles of 512
N_CORES = 8
NMR = 13           # staircase mask rank
QR = 64 + 1 + NMR  # q/k tile rows: 64 data + bias/ones + 13 mask rows

_CACHE = {}


def _split_multi_waits(nc, maxw=1):
    """walrus in this container rejects >1 sync wait per instruction;
    split extra waits onto preceding NOPs on the same engine."""
    import concourse.mybir as mybir
    for f in nc.m.functions:
        for bb in f.blocks:
            newlist = []
            for ins in bb.instructions:
                si = ins.sync_info
                if si is not None and si.on_wait and len(si.on_wait) > maxw:
                    waits = list(si.on_wait)
                    chunks = [waits[i:i + maxw] for i in range(0, len(waits), maxw)]
                    for ch in chunks[:-1]:
                        newlist.append(mybir.InstNoOp(
                            name=f"WSPLIT-{nc.next_id()}",
                            engine=ins.engine,
                            sync_info=mybir.SyncInfo(on_wait=list(ch), on_update=[]),
                            text_hint="wait_split",
                        ))
                    ins.sync_info = mybir.SyncInfo(
                        on_wait=list(chunks[-1]), on_update=list(si.on_update))
                newlist.append(ins)
            bb.instructions = newlist
    return nc


def build_program():
    import concourse.bass as bass
    import concourse.mybir as mybir
    import concourse.tile as tile

    f32 = mybir.dt.float32
    bf16 = mybir.dt.bfloat16
    AF = mybir.ActivationFunctionType

    nc = bass.Bass()
    xT = nc.dram_tensor("xT", [C, T], bf16, kind="ExternalInput")
    wqk = nc.dram_tensor("wqk", [C, 384], bf16, kind="ExternalInput")
    cq = nc.dram_tensor("cq", [NMR + 1, T], bf16, kind="ExternalInput")
    ck = nc.dram_tensor("ck", [HPG, NMR + 1, T], bf16, kind="ExternalInput")
    wv = nc.dram_tensor("wv", [C, CW], bf16, kind="ExternalInput")
    wp01 = nc.dram_tensor("wp01", [128, C], bf16, kind="ExternalInput")
    wp2 = nc.dram_tensor("wp2", [64, C], bf16, kind="ExternalInput")
    out = nc.dram_tensor("out", [C, T], bf16, kind="ExternalOutput")
    rcpb = nc.dram_tensor("rcpb", [HPG * NQT, 512], f32)
    rcpb2 = nc.dram_tensor("rcpb2", [HPG * NQT, 512], f32)

    with tile.TileContext(nc) as tc:
        with tc.tile_pool(name="persist", bufs=1) as persist, \
             tc.tile_pool(name="work", bufs=2) as work, \
             tc.tile_pool(name="psum", bufs=2, space="PSUM") as psum:

            # ---------------- load inputs ----------------
            # batched multi-dim DMAs (one dispatch each): the sync queue
            # dispatches serially at ~0.6us per DMA, so fewer is faster.
            # order matters: the first proj matmul needs wqk + xt[qt0].
            wqk_sb = persist.tile([128, 6, 384], bf16)
            wv_sb = persist.tile([128, 6, CW], bf16)
            xt_sb = persist.tile([128, 6, T], bf16)       # x^T, 6 chunks of C

            def _xt_in(qt):
                return bass.AP(tensor=xT, offset=qt * 512,
                               ap=[[T, 128], [128 * T, 6], [1, 512]])
            nc.sync.dma_start(out=xt_sb[:, :, 0:512], in_=_xt_in(0))
            nc.sync.dma_start(
                out=wqk_sb,
                in_=bass.AP(tensor=wqk, offset=0,
                            ap=[[384, 128], [128 * 384, 6], [1, 384]]))
            nc.sync.dma_start(
                out=wv_sb,
                in_=bass.AP(tensor=wv, offset=0,
                            ap=[[CW, 128], [128 * CW, 6], [1, CW]]))
            for qt in range(1, NQT):
                nc.sync.dma_start(out=xt_sb[:, :, qt * 512:(qt + 1) * 512],
                                  in_=_xt_in(qt))
            wp01_sb = persist.tile([128, C], bf16)
            nc.sync.dma_start(out=wp01_sb, in_=wp01[:, :])
            wp2_sb = persist.tile([64, C], bf16)
            nc.sync.dma_start(out=wp2_sb, in_=wp2[:, :])

            # q' / k' tiles: rows 0:64 = projections (device), row 64 =
            # ones / bq.k bias row, rows 65:78 = staircase mask rows (host)
            qpr = [persist.tile([QR, T], bf16, name=f"qpr{h}")
                   for h in range(HPG)]
            kpr = [persist.tile([QR, T], bf16, name=f"kpr{h}")
                   for h in range(HPG)]
            for h in range(HPG):
                nc.sync.dma_start(out=qpr[h][64:QR, :], in_=cq[:, :])
                nc.sync.dma_start(out=kpr[h][64:QR, :], in_=ck[h])

            # v natural layout + ones column: per head h cols
            # [65h .. 65h+63] = V_h, col 65h+64 = 1.0
            v_sb = persist.tile([128, NKC, HPG * 65], bf16)
            v_r = v_sb.rearrange("p n (h c) -> p n h c", c=65)
            nc.vector.memset(v_r[:, :, :, 64], 1.0)

            ynorm01 = persist.tile([128, T], bf16)   # heads 0,1 normalized y
            ynorm2 = persist.tile([64, T], bf16)     # head 2

            # ---------------- phase 1: projections ----------------
            # qk groups g: 0=[Qh0|Qh1] 1=[Kh0|Kh1] 2=[Qh2|Kh2], each 128 wide
            # interleaved with v chunks to hide v's LDWEIGHTS-bound matmuls
            qk_dst = [(qpr[0], qpr[1]), (kpr[0], kpr[1]), (qpr[2], kpr[2])]
            for qt in range(NQT):
                for g in range(4):
                    tch = 4 * qt + g
                    do_qk = g < 3
                    if do_qk:
                        qk_ps = psum.tile([128, 512], f32, tag="pv", bufs=3,
                                          name="qk_ps")
                    v_ps = psum.tile([128, 512], f32, tag="pv", bufs=3,
                                     name="v_ps")
                    for c in range(6):
                        if do_qk:
                            nc.tensor.matmul(
                                qk_ps,
                                lhsT=wqk_sb[:, c, g * 128:(g + 1) * 128],
                                rhs=xt_sb[:, c, qt * 512:(qt + 1) * 512],
                                start=(c == 0), stop=(c == 5))
                        nc.tensor.matmul(
                            v_ps[:, :CW],
                            lhsT=xt_sb[:, c, tch * 128:(tch + 1) * 128],
                            rhs=wv_sb[:, c, :],
                            start=(c == 0), stop=(c == 5))
                    if do_qk:
                        d0, d1 = qk_dst[g]
                        nc.vector.tensor_copy(
                            d0[0:64, qt * 512:(qt + 1) * 512], qk_ps[0:64, :])
                        nc.vector.tensor_copy(
                            d1[0:64, qt * 512:(qt + 1) * 512], qk_ps[64:128, :])
                    nc.vector.tensor_copy(
                        v_r[:, tch, :, 0:64],
                        v_ps[:, :CW].rearrange("p (h c) -> p h c", h=HPG))

            # ---------------- phase 2: attention ----------------
            # Per (h, qt): sorted-causal visible kcs = 0 .. 4qt+3.
            # kc < 4qt fully visible; kc = 4qt+i diagonal with local query
            # offset 128i (mask rows used on the first 128 cols).
            pending = []

            def _emit_pj(qt_p):
                for m in range(6):
                    pj_ps = psum.tile([128, 512], f32, tag="pv", bufs=3,
                                      name="pj_ps")
                    nc.tensor.matmul(
                        pj_ps, lhsT=wp01_sb[:, m * 128:(m + 1) * 128],
                        rhs=ynorm01[:, qt_p * 512:(qt_p + 1) * 512],
                        start=True, stop=False)
                    nc.tensor.matmul(
                        pj_ps, lhsT=wp2_sb[:, m * 128:(m + 1) * 128],
                        rhs=ynorm2[:, qt_p * 512:(qt_p + 1) * 512],
                        start=False, stop=True)
                    pj_sb = work.tile([128, 512], bf16, tag="pj", bufs=4,
                                      name="pj_sb")
                    nc.vector.tensor_copy(pj_sb, pj_ps)
                    nc.gpsimd.dma_start(
                        out=out[m * 128:(m + 1) * 128,
                                qt_p * 512:(qt_p + 1) * 512],
                        in_=pj_sb)

            def _norm_a(item):
                """Copy av (psum) to SBUF -- frees the psum slot early --
                and kick the denominator row to DRAM + its [128,4] reshaped
                readback (the [1,512]-shaped DVE reciprocal costs ~3.3us,
                the [128,4] one ~0.2us)."""
                av_p, h_p, qt_p = item
                slot = h_p * NQT + qt_p
                av_sb = work.tile([65, 512], f32, tag="avs", bufs=4,
                                  name="av_sb")
                nc.vector.tensor_copy(av_sb, av_p)
                d128 = work.tile([128, 4], f32, tag="d128", bufs=3,
                                 name="d128")
                nc.gpsimd.dma_start(out=d128, in_=av_sb[64:65, :])
                return (av_sb, d128, h_p, qt_p)

            def _norm_b(item):
                av_sb, d128, h_p, qt_p = item
                slot = h_p * NQT + qt_p
                rcp = work.tile([128, 4], f32, tag="rcp4", bufs=3, name="rcp")
                nc.vector.reciprocal(rcp, d128)
                r4b = bass.AP(tensor=rcpb2, offset=slot * 512,
                              ap=[[4, 128], [1, 4]])
                nc.sync.dma_start(out=r4b, in_=rcp)
                return (av_sb, h_p, qt_p)

            def _norm_c(item):
                av_sb, h_p, qt_p = item
                slot = h_p * NQT + qt_p
                bc_sb = work.tile([64, 512], f32, tag="bc", bufs=3,
                                  name="bc_sb")
                bcast_in = bass.AP(tensor=rcpb2, offset=slot * 512,
                                   ap=[[0, 64], [1, 512]])
                nc.gpsimd.dma_start(out=bc_sb, in_=bcast_in)
                return (av_sb, bc_sb, h_p, qt_p)

            def _norm_d(item):
                av_sb, bc_sb, h_p, qt_p = item
                if h_p == 0:
                    dst = ynorm01[0:64, qt_p * 512:(qt_p + 1) * 512]
                elif h_p == 1:
                    dst = ynorm01[64:128, qt_p * 512:(qt_p + 1) * 512]
                else:
                    dst = ynorm2[:, qt_p * 512:(qt_p + 1) * 512]
                nc.vector.tensor_mul(dst, av_sb[0:64, :], bc_sb)
                return (h_p, qt_p)

            def _pop_pending():
                if not pending:
                    return
                stage, item = pending.pop(0)
                if stage == 0:
                    pending.append((1, _norm_a(item)))
                elif stage == 1:
                    pending.append((2, _norm_b(item)))
                elif stage == 2:
                    pending.append((3, _norm_c(item)))
                elif stage == 3:
                    h_p, qt_p = _norm_d(item)
                    if h_p == HPG - 1:
                        pending.append((4, qt_p))
                else:
                    _emit_pj(item)

            def _zb(j):
                return 128 * j - (128 * j) % 10

            for qt in range(NQT):
                q0 = qt * 512
                # regions: (kc, w0, w1, masked) with [w0, w1) window-relative.
                # Key chunk j: masked zone queries [zb(j), zb(j+1)), fully
                # visible queries >= zb(j+1), invisible below zb(j).
                regions = []
                for j in range(NKC + 1):
                    if j >= NKC or _zb(j) >= q0 + 512:
                        break
                    mlo = max(q0, _zb(j))
                    mhi = min(q0 + 512, _zb(j + 1))
                    if mhi > mlo:
                        regions.append((j, mlo - q0, mhi - q0, True))
                    flo = max(q0, _zb(j + 1))
                    if q0 + 512 > flo:
                        regions.append((j, flo - q0, 512, False))
                # pack into slabs of <=1024 sc cols, splitting at the 512-col
                # psum bank boundaries (a matmul may not cross a bank)
                slabs = []
                cur, cur_cols = [], 0
                for kc, w0, w1, masked in regions:
                    while w0 < w1:
                        if cur_cols == 1024:
                            slabs.append(cur)
                            cur, cur_cols = [], 0
                        bank_room = 512 - cur_cols % 512
                        take = min(w1 - w0, bank_room)
                        cur.append((kc, w0, w0 + take, cur_cols, masked))
                        cur_cols += take
                        w0 += take
                if cur:
                    slabs.append(cur)
                last_piece = slabs[-1][-1]
                pieces0 = slabs[0][0]

                for h in range(HPG):
                    av = psum.tile([65, 512], f32, tag="pv", bufs=3, name="av")
                    av_todo = []

                    def _emit_av(args, h=h, av=av, p0=pieces0,
                                 pl=last_piece):
                        pt, pieces = args
                        for piece in pieces:
                            kc, w0, w1, soff, masked = piece
                            nc.tensor.matmul(
                                av[:, w0:w1],
                                lhsT=v_sb[:, kc, 65 * h:65 * h + 65],
                                rhs=pt[:, soff:soff + (w1 - w0)],
                                start=(piece == p0),
                                stop=(piece == pl),
                                skip_group_check=True)

                    def _emit_slab(pieces, h=h):
                        sc = psum.tile([128, 1024], f32, tag="sc", name="sc")
                        total = 0
                        bank_started = set()
                        for kc, w0, w1, soff, masked in pieces:
                            kcol = kc * 128
                            rows = QR if masked else 65
                            st = (soff // 512) not in bank_started
                            bank_started.add(soff // 512)
                            nc.tensor.matmul(
                                sc[:, soff:soff + (w1 - w0)],
                                lhsT=kpr[h][0:rows, kcol:kcol + 128],
                                rhs=qpr[h][0:rows, q0 + w0:q0 + w1],
                                start=st, stop=True,
                                skip_group_check=True)
                            total = soff + (w1 - w0)
                        pt = work.tile([128, 1024], bf16, tag="pt", bufs=3,
                                       name="pt")
                        nc.scalar.activation(pt[:, :total], sc[:, :total],
                                             AF.Exp, scale=0.125)
                        return pt

                    for pieces in slabs:
                        _pop_pending()
                        pt = _emit_slab(pieces)
                        av_todo.append((pt, pieces))
                        if len(av_todo) > 1:
                            _emit_av(av_todo.pop(0))
                    while av_todo:
                        _emit_av(av_todo.pop(0))
                    pending.append((0, (av, h, qt)))
            while pending:
                _pop_pending()

    _split_multi_waits(nc)
    return nc


def get_program():
    if "nc" not in _CACHE:
        _CACHE["nc"] = build_program()
    return _CACHE["nc"]


def _perm():
    # device position s <-> original position perm[s] = (s%10)*256 + s//10
    s = np.arange(T)
    return (s % 10) * 256 + s // 10


def _mask_rows():
    """A rows (key side, carry -1e9, base = own 128-chunk) and B rows
    (query side 0/1, base = the key chunk whose visibility zone holds q)."""
    s = np.arange(T)
    r = s // 10                        # residue of sorted position
    baseA = (128 * (s // 128)) // 10   # key chunk base residue
    locA = r - baseA
    z = (10 * (s // 10) + 9) // 128    # zone: key chunk containing q's bound
    baseB = (128 * z) // 10
    locB = r - baseB
    ells = np.arange(1, NMR + 1)[:, None]
    A = np.where(locA[None, :] >= ells, np.float32(-1e9), np.float32(0.0))
    Bm = (locB[None, :] < ells).astype(np.float32)
    return A, Bm


def make_in_maps(x, Wk, bk, Wq, bq, Wv, bv, Wp, bp):
    import ml_dtypes
    b16 = ml_dtypes.bfloat16
    x = np.asarray(x, dtype=np.float32)
    Wk = np.asarray(Wk, dtype=np.float32)
    Wq = np.asarray(Wq, dtype=np.float32)
    Wv = np.asarray(Wv, dtype=np.float32)
    Wp = np.asarray(Wp, dtype=np.float32)
    bq_f = np.asarray(bq, dtype=np.float32)
    perm = _perm()
    A, Bm = _mask_rows()
    cq = np.concatenate([np.ones((1, T), np.float32), Bm], axis=0)

    in_maps = []
    for core in range(N_CORES):
        b, g = divmod(core, 4)
        h0 = g * HPG
        r = slice(h0 * HD, (h0 + HPG) * HD)      # 192 head dims
        xs = x[b][perm]                          # sorted positions [T, C]
        xt = np.ascontiguousarray(xs.T)
        wq_g = Wq[r]                             # [192, 768]
        wk_g = Wk[r]
        # wqk cols: [Qh0|Qh1(128) | Kh0|Kh1(128) | Qh2|Kh2(128)]
        wqk = np.concatenate(
            [wq_g[:128].T, wk_g[:128].T, wq_g[128:].T, wk_g[128:].T],
            axis=1).astype(np.float32)
        # k-side extra rows: row 0 = bq.k_j = xs @ (Wk_h^T bq_h), rows 1:14 = A
        ck = np.empty((HPG, NMR + 1, T), np.float32)
        for h in range(HPG):
            hb = slice((h0 + h) * HD, (h0 + h + 1) * HD)
            khat = xs @ (Wk[hb].T @ bq_f[hb])    # [T]
            ck[h, 0] = khat
            ck[h, 1:] = A
        wv_g = np.ascontiguousarray(Wv[r].T).astype(np.float32)
        wp_g = Wp[:, r]                          # [768, 192]
        wp01 = np.ascontiguousarray(wp_g[:, 0:128].T)
        wp2 = np.ascontiguousarray(wp_g[:, 128:192].T)
        in_maps.append({
            "xT": np.ascontiguousarray(xt).astype(b16),
            "wqk": np.ascontiguousarray(wqk).astype(b16),
            "cq": cq.astype(b16),
            "ck": ck.astype(b16),
            "wv": wv_g.astype(b16),
            "wp01": wp01.astype(b16),
            "wp2": wp2.astype(b16),
        })
    return in_maps


def kernel(x, Wk, bk, Wq, bq, Wv, bv, Wp, bp):
    from concourse.bass_utils import run_bass_kernel_spmd
    nc = get_program()
    in_maps = make_in_maps(x, Wk, bk, Wq, bq, Wv, bv, Wp, bp)
    res = run_bass_kernel_spmd(nc, in_maps, list(range(N_CORES)))
    Wp_np = np.asarray(Wp, dtype=np.float32)
    const = (np.asarray(bp, dtype=np.float32)
             + Wp_np @ np.asarray(bv, dtype=np.float32))   # [768]
    perm = _perm()
    out = np.empty((B, T, C), dtype=np.float32)
    for b in range(B):
        acc = res.results[b * 4 + 0]["out"].astype(np.float32).copy()
        for g in range(1, 4):
            acc += res.results[b * 4 + g]["out"]
        out[b][perm] = acc.T + const[None, :]
    return out
